# revision 1
# baseline (speedup 1.0000x reference)
"""Trainium2 Bass kernel for a vanilla tanh RNN (last hidden state).

    h_t = tanh(x_t @ W + h_{t-1} @ U + b),  h_0 = 0,  return h_T  [B, H]

Shapes: x [64, 1024, 1024] f32, W/U [1024, 1024] f32, b [1024] f32.
Data-parallel over batch: 8 cores x 8 rows each; W/U/b replicated.

Precision strategy (error budget 2e-2 absmax-rel):
  - Phase A (x @ W) runs in float32r: single product at bf16 speed for
    free-dim >= 256, measured ~1.7e-4 matmul error (vs 2.5e-3 for bf16).
  - The recurrence keeps U and h in bf16 (dominant error term, ~8e-3 on
    the final output).

Per-core program (SPMD, identical NEFF):
  Phase A: xwT[n*128+p, hc, c] = (x @ W + b)[b, t, hc*128+p] -> DRAM f32,
           with t = n*64 + c//8, b = c%8.  x tiles are PE-transposed
           (f32r pass-through, exact) into (t,b)-interleaved column order.
  Phase B: 1024 sequential steps in [H-part, B-free] orientation:
           one PSUM tile [128, 64] per step; 64 bf16 U-stationary matmuls
           (N=8) in k-split order (k=0..3 for all m, then k=4..7) so the
           chunks of h_t are produced early enough for step t+1's
           consumers (~1us of slack vs ~0.26us with m-major order).
           Then per m-pair: DVE adds xw_t into PSUM, ACT writes
           tanh -> h bf16.
  Epilogue: PE-transpose h.T back to [B, H] and DMA out.

The LDW+MM pair rate at N=8 is ~35 ns on HW (weight-load bound: all of U
re-streams through the PE every step), so phase B's floor is
64*35ns*1024 ~= 2.3 ms; phase A adds ~0.3 ms.
"""

import os
import sys
import time
from contextlib import ExitStack

import numpy as np

sys.path.insert(0, "/opt/trn_rl_repo")

B_FULL, T, D, H = 64, 1024, 1024, 1024
NCORES = 8
B = B_FULL // NCORES  # 8 rows per core
KC = D // 128  # 8 contraction chunks
MC = H // 128  # 8 output chunks
NBLK = T // 64  # 16 col-blocks of 64 timesteps (512 cols each)


def build_bass(nsteps=T, reps=1, debug_xw=False, nblk_a=NBLK, xw_mode="f32r"):
    import concourse.bacc as bacc
    import concourse.bass as bass
    import concourse.mybir as mybir
    import concourse.tile as tile
    from concourse.masks import make_identity

    f32 = mybir.dt.float32
    f32r = mybir.dt.float32r
    bf16 = mybir.dt.bfloat16
    TANH = mybir.ActivationFunctionType.Tanh
    IDENT = mybir.ActivationFunctionType.Identity

    nc = bacc.Bacc(
        "TRN2", target_bir_lowering=False, debug=False, num_devices=NCORES
    )

    x = nc.dram_tensor("x", [B, T, D], f32, kind="ExternalInput")
    Wt = nc.dram_tensor("W", [D, H], f32, kind="ExternalInput")
    Ut = nc.dram_tensor("U", [H, H], f32, kind="ExternalInput")
    bt = nc.dram_tensor("b", [H], f32, kind="ExternalInput")
    out = nc.dram_tensor("out", [B, H], f32, kind="ExternalOutput")
    xwT = nc.dram_tensor(
        "xwT", [NBLK * 128, MC, 512], f32,
        kind="ExternalOutput" if debug_xw else "Internal",
    )

    nblk_run = 0 if debug_xw else max(1, min(NBLK, nsteps // 64))

    with tile.TileContext(nc) as tc, ExitStack() as ctx:
        const_pool = ctx.enter_context(tc.tile_pool(name="const", bufs=1))
        if xw_mode == "f32r":
            W_sb = const_pool.tile([128, KC, H], f32r)  # [p(d), dc, h] 32KB/p
        else:  # hilo2: bf16 hi/lo split of W, x rounded to bf16
            W_hi = const_pool.tile([128, KC, H], bf16)
            W_lo = const_pool.tile([128, KC, H], bf16)
        U_sb = const_pool.tile([128, KC, H], bf16)  # [p(k), kc, m] 16KB/p
        b_sb = const_pool.tile([128, MC], f32)
        idb = const_pool.tile([128, 128], bf16)
        idf = const_pool.tile([128, 128], f32)
        idr = const_pool.tile([128, 128], f32r)
        hT_a = const_pool.tile([128, MC * B], bf16)  # [p(h), m*8+b]
        hT_b = const_pool.tile([128, MC * B], bf16)

        xin_pool = ctx.enter_context(tc.tile_pool(name="xin", bufs=3))
        xt_pool = ctx.enter_context(tc.tile_pool(name="xt", bufs=2))
        xwsb_pool = ctx.enter_context(tc.tile_pool(name="xwsb", bufs=3))
        bpool = ctx.enter_context(tc.tile_pool(name="xwblk", bufs=2))

        make_identity(nc, idb[:])
        make_identity(nc, idf[:])
        nc.vector.tensor_copy(idr[:], idf[:])  # f32r producers must round
        nc.sync.dma_start(b_sb[:], bt.ap().rearrange("(hc p) -> p hc", p=128))
        # stage W/U through bpool tiles: pool-internal rotation gives the
        # later xwblk DMA writes proper WAR deps on these tiles' readers
        # (a freed dedicated pool would NOT be ordered against new tenants)
        stage = bpool.tile([128, KC, H], f32, name="xwblk")
        nc.sync.dma_start(
            stage[:], Ut.ap().rearrange("(kc p) m -> p kc m", p=128)
        )
        nc.vector.tensor_copy(U_sb[:], stage[:])
        stage2 = bpool.tile([128, KC, H], f32, name="xwblk")
        nc.sync.dma_start(
            stage2[:], Wt.ap().rearrange("(dc p) h -> p dc h", p=128)
        )
        if xw_mode == "f32r":
            nc.vector.tensor_copy(W_sb[:], stage2[:])
        else:
            nc.vector.tensor_copy(W_hi[:], stage2[:])  # round to bf16
            nc.vector.tensor_copy(stage[:], W_hi[:])  # back to f32
            nc.vector.tensor_sub(stage[:], stage2[:], stage[:])
            nc.vector.tensor_copy(W_lo[:], stage[:])
        actx = ExitStack()
        tps_pool = actx.enter_context(
            tc.tile_pool(name="tps", bufs=2, space="PSUM")
        )
        gps_pool = actx.enter_context(
            tc.tile_pool(name="gps", bufs=2, space="PSUM")
        )

        # ---------------- Phase A: xwT = (x @ W + b).T, cols = t*8+b --------
        for n in range(nblk_a):
            xt_dt = f32r if xw_mode == "f32r" else bf16
            xt = xt_pool.tile([128, KC, 512], xt_dt)  # [p(d), dc, col]
            for q in range(4):  # 16 timesteps per transpose tile
                t0 = n * 64 + q * 16
                xin = xin_pool.tile([128, D], f32)  # [(t b), d]
                xinv = xin[:].rearrange("(t b) d -> t b d", t=16)
                for bb in range(B):
                    nc.sync.dma_start(
                        xinv[:, bb, :], x.ap()[bb, t0 : t0 + 16, :]
                    )
                xin_r = xin_pool.tile([128, D], xt_dt, name="xin_r")
                nc.vector.tensor_copy(xin_r[:], xin[:])  # round
                idx = idr if xw_mode == "f32r" else idb
                for dc in range(KC):
                    tps = tps_pool.tile([128, 128], xt_dt)
                    nc.tensor.transpose(
                        tps[:], xin_r[:, dc * 128 : (dc + 1) * 128], idx[:]
                    )
                    nc.scalar.copy(xt[:, dc, q * 128 : (q + 1) * 128], tps[:])
            for hc in range(MC):
                ps = gps_pool.tile([128, 512], f32)
                if xw_mode == "f32r":
                    for dc in range(KC):
                        nc.tensor.matmul(
                            ps[:],
                            W_sb[:, dc, hc * 128 : (hc + 1) * 128],
                            xt[:, dc, :],
                            start=(dc == 0),
                            stop=(dc == KC - 1),
                        )
                else:
                    for wi, wt in enumerate((W_hi, W_lo)):
                        for dc in range(KC):
                            nc.tensor.matmul(
                                ps[:],
                                wt[:, dc, hc * 128 : (hc + 1) * 128],
                                xt[:, dc, :],
                                start=(wi == 0 and dc == 0),
                                stop=(wi == 1 and dc == KC - 1),
                            )
                xw_sb = xwsb_pool.tile([128, 512], f32)
                nc.scalar.activation(
                    xw_sb[:], ps[:], IDENT, bias=b_sb[:, hc : hc + 1], scale=1.0
                )
                nc.sync.dma_start(
                    xwT.ap()[n * 128 : (n + 1) * 128, hc, :], xw_sb[:]
                )

        if xw_mode == "f32r":
            # f32r -> bf16 weight-path fence: walrus only arms its FWL
            # hang-workaround after *fp32* matmuls, so run one tiny plain-fp32
            # matmul before the first bf16 FWL load of phase B.  Reading the
            # last xw_sb tile data-orders it after the f32r stream.
            fps = tps_pool.tile([128, 1], f32, name="fence")
            nc.tensor.matmul(
                fps[:], xw_sb[:, :128], xw_sb[:, :1], start=True, stop=True
            )

        # ---------------- Phase B: the recurrence ---------------------------
        actx.close()  # free phase-A PSUM: phase B needs all 8 banks
        zctx = ExitStack()
        # one full 2KB PSUM bank (= one zero region) per accumulation group:
        # 4 groups per wave are open concurrently and MUST NOT share a region.
        # Each wave's tile spans 4 banks; bufs=2 double-buffers across waves.
        pspool = zctx.enter_context(tc.tile_pool(name="zps", bufs=2, space="PSUM"))

        rctx = ExitStack()
        if reps != 1:
            rctx.enter_context(tc.For_i(0, reps))
        nc.gpsimd.memset(hT_a[:], 0.0)
        bctx = ExitStack()
        if nblk_run > 0:
            blk = bctx.enter_context(
                tc.For_i(0, nblk_run, hint_engines=(mybir.EngineType.PE,))
            )
            xwblk = bpool.tile([128, MC, 512], f32, name="xwblk")
            nc.sync.dma_start(xwblk[:], xwT.ap()[bass.ts(blk, 128), :, :])
            for s in range(64):
                src = hT_a if s % 2 == 0 else hT_b
                dst = hT_b if s % 2 == 0 else hT_a
                dst_v = dst[:].rearrange("p (m b) -> p m b", b=B)
                # two waves of 4 m-groups; within each wave, k is split in
                # halves so step t's chunks close with >=16 pairs of slack
                # before step t+1 consumes them
                for wave in (0, 1):
                    msl = slice(wave * 4, wave * 4 + 4)
                    ps = pspool.tile([128, 4, 512], f32)  # 4 banks, 1/group
                    for kh in (0, 1):
                        for mi in range(4):
                            m = wave * 4 + mi
                            for k in range(kh * 4, kh * 4 + 4):
                                nc.tensor.matmul(
                                    ps[:, mi, :B],
                                    U_sb[:, k, m * 128 : (m + 1) * 128],
                                    src[:, k * B : (k + 1) * B],
                                    start=(k == 0),
                                    stop=(k == KC - 1),
                                )
                    nc.vector.tensor_add(
                        ps[:, :, :B], ps[:, :, :B],
                        xwblk[:, msl, s * B : (s + 1) * B],
                    )
                    nc.scalar.activation(dst_v[:, msl, :], ps[:, :, :B], TANH)
        bctx.close()
        rctx.close()
        zctx.close()

        # ---------------- Epilogue: h.T -> out [B, H] -----------------------
        ops_pool = ctx.enter_context(tc.tile_pool(name="ops", bufs=2, space="PSUM"))
        osb_pool = ctx.enter_context(tc.tile_pool(name="osb", bufs=2))
        for m in range(MC):
            tp = ops_pool.tile([B, 128], bf16)
            nc.tensor.transpose(tp[:], hT_a[:, m * B : (m + 1) * B], idb[:])
            ob = osb_pool.tile([B, 128], f32)
            nc.scalar.copy(ob[:], tp[:])
            nc.sync.dma_start(out.ap()[:, m * 128 : (m + 1) * 128], ob[:])

    nc.compile()
    return nc


_NC_CACHE = None


def _in_maps(x, W, U, b):
    return [
        {"x": x[i * B : (i + 1) * B], "W": W, "U": U, "b": b}
        for i in range(NCORES)
    ]


def kernel(**inputs: np.ndarray) -> np.ndarray:
    global _NC_CACHE
    from concourse.bass_utils import run_bass_kernel_spmd

    x = np.ascontiguousarray(inputs["x"], dtype=np.float32)
    W = np.ascontiguousarray(inputs["W"], dtype=np.float32)
    U = np.ascontiguousarray(inputs["U"], dtype=np.float32)
    b = np.ascontiguousarray(inputs["b"], dtype=np.float32)

    if _NC_CACHE is None:
        _NC_CACHE = build_bass()
    nc = _NC_CACHE

    res = run_bass_kernel_spmd(nc, _in_maps(x, W, U, b), core_ids=list(range(NCORES)))
    outs = [res.results[i]["out"] for i in range(NCORES)]
    return np.concatenate(outs, axis=0).astype(np.float32)


# ---------------- HW timing (used by test.py; PJRT-level, resident inputs) --


def _make_runner(nc, n_cores=NCORES):
    import jax
    from jax.sharding import Mesh, NamedSharding, PartitionSpec

    try:
        from jax.experimental.shard_map import shard_map
    except ImportError:
        from jax import shard_map
    from concourse import bass2jax, mybir

    bass2jax.install_neuronx_cc_hook()
    partition_name = nc.partition_id_tensor.name if nc.partition_id_tensor else None

    in_names, out_names, out_avals, zero_outs = [], [], [], []
    for alloc in nc.m.functions[0].allocations:
        if not isinstance(alloc, mybir.MemoryLocationSet):
            continue
        name = alloc.memorylocations[0].name
        if alloc.kind == "ExternalInput":
            if name != partition_name:
                in_names.append(name)
        elif alloc.kind == "ExternalOutput":
            out_names.append(name)
            shape = tuple(alloc.tensor_shape)
            dtype = mybir.dt.np(alloc.dtype)
            out_avals.append(jax.core.ShapedArray(shape, dtype))
            zero_outs.append(np.zeros(shape, dtype))
    n_params = len(in_names)
    all_in_names = list(in_names) + list(out_names)
    if partition_name is not None:
        all_in_names.append(partition_name)

    def _body(*args):
        operands = list(args)
        if partition_name is not None:
            operands.append(bass2jax.partition_id_tensor())
        return tuple(
            bass2jax._bass_exec_p.bind(
                *operands,
                out_avals=tuple(out_avals),
                in_names=tuple(all_in_names),
                out_names=tuple(out_names),
                lowering_input_output_aliases=(),
                sim_require_finite=True,
                sim_require_nnan=True,
                nc=nc,
            )
        )

    devices = jax.devices()[:n_cores]
    mesh = Mesh(np.asarray(devices), ("core",))
    nio = n_params + len(out_names)
    fn = jax.jit(
        shard_map(
            _body,
            mesh=mesh,
            in_specs=(PartitionSpec("core"),) * nio,
            out_specs=(PartitionSpec("core"),) * len(out_names),
            check_rep=False,
        ),
        keep_unused=True,
    )
    sharding = NamedSharding(mesh, PartitionSpec("core"))

    def prepare(in_maps):
        concat = [
            np.concatenate(
                [np.asarray(in_maps[c][nm]) for c in range(n_cores)], axis=0
            )
            for nm in in_names
        ] + [
            np.zeros((n_cores * z.shape[0], *z.shape[1:]), z.dtype)
            for z in zero_outs
        ]
        dev = [jax.device_put(a, sharding) for a in concat]
        jax.block_until_ready(dev)
        return dev

    return fn, prepare, out_names, out_avals


def _bench_nc(nc, in_maps, warmup=2, iters=8):
    import jax

    fn, prepare, out_names, out_avals = _make_runner(nc)
    dev_args = prepare(in_maps)
    for _ in range(warmup):
        jax.block_until_ready(fn(*dev_args))
    times = []
    for _ in range(iters):
        t0 = time.perf_counter()
        outs = fn(*dev_args)
        jax.block_until_ready(outs)
        times.append(time.perf_counter() - t0)
    res = []
    for c in range(NCORES):
        res.append(
            {
                nm: np.asarray(outs[i]).reshape(NCORES, *out_avals[i].shape)[c]
                for i, nm in enumerate(out_names)
            }
        )
    return min(times), res


def build_trivial():
    """Same I/O signature, no compute: measures pure per-call overhead."""
    import concourse.bacc as bacc
    import concourse.mybir as mybir
    import concourse.tile as tile

    f32 = mybir.dt.float32
    nc = bacc.Bacc(
        "TRN2", target_bir_lowering=False, debug=False, num_devices=NCORES
    )
    nc.dram_tensor("x", [B, T, D], f32, kind="ExternalInput")
    nc.dram_tensor("W", [D, H], f32, kind="ExternalInput")
    nc.dram_tensor("U", [H, H], f32, kind="ExternalInput")
    nc.dram_tensor("b", [H], f32, kind="ExternalInput")
    out = nc.dram_tensor("out", [B, H], f32, kind="ExternalOutput")
    with tile.TileContext(nc) as tc, ExitStack() as ctx:
        pool = ctx.enter_context(tc.tile_pool(name="o", bufs=1))
        ob = pool.tile([B, H], f32)
        nc.gpsimd.memset(ob[:], 0.0)
        nc.sync.dma_start(out.ap(), ob[:])
    nc.compile()
    return nc


def measure_hw_time(inputs, reps=9):
    """Return (total_exec_ns, phase_b_ns, outputs_of_reps_run).

    total = t(reps=1) - t(trivial kernel)   [call overhead subtracted]
    phase_b = (t(reps=R) - t(reps=1)) / (R-1)   [reps loop wraps phase B]
    """
    x = np.ascontiguousarray(inputs["x"], dtype=np.float32)
    maps = _in_maps(
        x,
        np.ascontiguousarray(inputs["W"], dtype=np.float32),
        np.ascontiguousarray(inputs["U"], dtype=np.float32),
        np.ascontiguousarray(inputs["b"], dtype=np.float32),
    )
    t0, _ = _bench_nc(build_trivial(), maps)
    t1, _ = _bench_nc(build_bass(reps=1), maps)
    tR, res = _bench_nc(build_bass(reps=reps), maps)
    per_rep = (tR - t1) / (reps - 1)
    total = t1 - t0
    full = np.concatenate([res[i]["out"] for i in range(NCORES)], axis=0)
    return total * 1e9, per_rep * 1e9, full.astype(np.float32)


if __name__ == "__main__":
    rng = np.random.default_rng(0)
    ins = {
        "x": rng.standard_normal((B_FULL, T, D), dtype=np.float32),
        "W": (rng.standard_normal((D, H), dtype=np.float32) / np.sqrt(D)),
        "U": (rng.standard_normal((H, H), dtype=np.float32) / np.sqrt(H)),
        "b": np.zeros((H,), dtype=np.float32),
    }
    got = kernel(**ins)
    print("out", got.shape, got.dtype)

